# revision 8
# baseline (speedup 1.0000x reference)
"""Trainium2 Bass kernel for nn_ChannelWisePatchLevelObfuscator.

Math: split each (512,512) image into 32x32 patches of 16x16; per (channel,
group) apply a dense 256->256 obfuscation matmul over patch pixels (group =
(row+col) % 32), add bias, tanh, then permute channels.

Sharding: model-parallel over the 96 (channel, group) pairs -- 12 pairs per
core, each pair covering the FULL batch (T = B*NH = 2048 matmul rows). Unlike
batch-parallel sharding (which replicates all 12 MiB of fp16 weights on every
core), this loads each weight exactly once chip-wide: per-core traffic drops
from 36 MiB (12 x + 12 w + 12 out) to 25.5 MiB (12 x + 1.5 w + 12 out).

Layout strategy: the host packs x into a pair-major, contraction-major
("pixel on partition") slab layout and pre-permutes W/bias to match, so every
device DMA is a fully-contiguous [128 x 8KiB-per-partition] transfer at peak
HBM bandwidth. The channel permutation is applied for free during the host
unpack scatter.

Device loop per core: all weights (12 x [128,512] fp16 = 1.5 MiB) preloaded
into SBUF once. Per pair: one 1 MiB x DMA; per output half oc, 8 matmuls
(stationary W[kc,oc] streamed over 4 t-tiles of 512, K accumulated over 2
chunks) fill a 4-bank PSUM tile [128,2048]; ONE ScalarE activation then does
bias + tanh + PSUM->SBUF fp16 for the whole 2048-wide tile (4x fewer ACT
instructions than per-bank activations -- ScalarE was the covert near-
bottleneck of the batch-parallel kernel at 97/112 us busy).

Precision: matmul inputs and the tanh output are fp16 (fp32 PSUM accumulate);
rel err vs fp32 reference ~3.6e-4.
"""
import sys
import numpy as np

sys.path.insert(0, "/opt/trn_rl_repo")

import concourse.bacc as bacc  # noqa: E402
import concourse.mybir as mybir  # noqa: E402
import concourse.tile as tile  # noqa: E402
from concourse.bass_utils import run_bass_kernel_spmd  # noqa: E402

IMG, C, PS, G, B = 512, 3, 16, 32, 64
NH = NW = IMG // PS          # 32 patches per side
P2 = PS * PS                 # 256 pixels per patch
NCORES = 8
NPAIR = C * G                # 96 (channel, group) pairs
PPC = NPAIR // NCORES        # 12 pairs per core
TF = B * NH                  # 2048 matmul rows per pair (full batch)
NT = TF // 512               # 4 moving tiles of 512 per (kc, oc)

F32 = mybir.dt.float32
MM_DT = mybir.dt.float16     # matmul input dtype
OUT_DT = mybir.dt.float16    # device store dtype; host upcasts to fp32
NP_MM = np.float16

_g = np.arange(G)[:, None]
_r = np.arange(NH)[None, :]
COLS = (_g - _r) % NW        # (g, r) -> patch column belonging to group g

_CACHE = {}


def _build_nc():
    nc = bacc.Bacc("TRN2", target_bir_lowering=False, debug=False,
                   num_devices=NCORES)
    # slab layouts: per-pair x/out tiles are [128, 8 KiB-per-partition]
    # fully-contiguous transfers; w/bias load once up front.
    xt = nc.dram_tensor("xt", [PPC, 2, 128, TF], MM_DT, kind="ExternalInput")
    w = nc.dram_tensor("w", [PPC, 128, 512], MM_DT, kind="ExternalInput")
    bias = nc.dram_tensor("bias", [128, PPC * 2], F32, kind="ExternalInput")
    out = nc.dram_tensor("out", [PPC, 2, 128, TF], OUT_DT,
                         kind="ExternalOutput")

    with tile.TileContext(nc) as tc:
        with tc.tile_pool(name="biasp", bufs=1) as bias_pool, \
             tc.tile_pool(name="wp", bufs=PPC) as w_pool, \
             tc.tile_pool(name="xtp", bufs=2 * PPC) as xt_pool, \
             tc.tile_pool(name="outp", bufs=8) as out_pool, \
             tc.tile_pool(name="psp", bufs=2, space="PSUM") as ps_pool:
            # Tile dependencies gate at whole-tile granularity, so weights
            # are PER-PAIR tiles and x is per-(pair, kc-half) tiles. ALL x
            # tiles stay resident (24 x 4 KiB/partition) so the load stream
            # is never slot-gated: it front-runs at full HBM rate while the
            # store ring is still empty, which keeps PE gap-free (HAM warm).
            # Everything inbound rides the SP ring in pair order (w[j] just
            # ahead of its x halves); the ACT ring carries only stores.
            bias_sb = bias_pool.tile([128, PPC * 2], F32)
            nc.sync.dma_start(bias_sb[:], bias[:, :])
            w_t = []
            xk = []
            for j in range(PPC):
                wt = w_pool.tile([128, 512], MM_DT)
                nc.sync.dma_start(wt[:], w[j])
                w_t.append(wt)
                for kc in range(2):
                    xt_t = xt_pool.tile([128, TF], MM_DT)
                    nc.sync.dma_start(xt_t[:], xt[j, kc])
                    xk.append(xt_t)
            for j in range(PPC):
                for oc in range(2):
                    ps = ps_pool.tile([128, TF], F32)  # 4 PSUM banks
                    for kc in range(2):
                        wof = kc * 256 + oc * 128
                        for tt in range(NT):
                            nc.tensor.matmul(
                                ps[:, tt * 512:(tt + 1) * 512],
                                w_t[j][:, wof:wof + 128],
                                xk[2 * j + kc][:, tt * 512:(tt + 1) * 512],
                                start=(kc == 0), stop=(kc == 1))
                    out_t = out_pool.tile([128, TF], OUT_DT)
                    bidx = j * 2 + oc
                    nc.scalar.activation(
                        out_t[:],
                        ps[:],
                        mybir.ActivationFunctionType.Tanh,
                        bias=bias_sb[:, bidx:bidx + 1],
                        scale=1.0)
                    nc.scalar.dma_start(out[j, oc], out_t[:])
    nc.compile()
    return nc


def _pack_xt_all(x):
    # (B, C, 512, 512) -> [96, 256, 2048] fp16: xt_all[pair, p, t] with
    # p = py*16+px on what becomes the partition axis, t = b*32 + r
    xp = x.reshape(B, C, NH, PS, NW, PS)               # b c r py cl px
    sel = xp[:, :, _r, :, COLS, :]                     # g r b c py px
    xt_all = sel.transpose(3, 0, 4, 5, 2, 1)           # c g py px b r
    return xt_all.reshape(NPAIR, P2, TF).astype(NP_MM)


def _pack_xt_core(xt_all, core):
    # [12, 256, 2048] -> [12, kc, 128, t] per-(pair, kc-half) slabs
    sl = xt_all[core * PPC:(core + 1) * PPC]
    return np.ascontiguousarray(sl.reshape(PPC, 2, 128, TF))


def _pack_w_core(w_full, core):
    # [c,g,p_in,p_out] -> [j, 128 k_lo, kc*256 + oc*128 + o_lo]
    sel = (w_full.reshape(NPAIR, P2, P2)[core * PPC:(core + 1) * PPC]
           .astype(NP_MM))
    sel = sel.reshape(PPC, 2, 128, 2, 128).transpose(0, 2, 1, 3, 4)
    return np.ascontiguousarray(sel.reshape(PPC, 128, 512))


def _pack_bias_core(b_full, core):
    # [c,g,o] -> [128 o_lo, j*2 + oc] fp32
    sel = b_full.reshape(NPAIR, P2)[core * PPC:(core + 1) * PPC]
    sel = sel.reshape(PPC, 2, 128).transpose(2, 0, 1)
    return np.ascontiguousarray(sel.reshape(128, PPC * 2))


def _unpack_core(out_dev):
    # [12, oc, 128, t] fp16 -> [12, 256, B, NH] f32 (o = oc*128+o_lo)
    return out_dev.astype(np.float32).reshape(PPC, P2, B, NH)


def _assemble(o_all, perm):
    # o_all [96, 256, B, NH] -> full (B, C, 512, 512) with channel perm
    src = (o_all.reshape(C, G, PS, PS, B, NH)
           .transpose(1, 5, 4, 0, 2, 3))               # g r b c py px
    tmp = np.empty((NH, NW, B, C, PS, PS), dtype=np.float32)
    tmp[_r, COLS] = src                                # tmp[r, (g-r)%32]
    img = tmp.transpose(2, 3, 0, 4, 1, 5).reshape(B, C, IMG, IMG)
    return np.ascontiguousarray(img[:, perm])


def kernel(x, obfuscation_weights, obfuscation_biases, channel_permutation):
    x = np.ascontiguousarray(x, dtype=np.float32)
    w = np.ascontiguousarray(obfuscation_weights, dtype=np.float32)
    bias = np.asarray(obfuscation_biases, dtype=np.float32)
    perm = np.asarray(channel_permutation, dtype=np.int64)

    if "nc" not in _CACHE:
        _CACHE["nc"] = _build_nc()
    nc = _CACHE["nc"]

    xt_all = _pack_xt_all(x)
    in_maps = []
    for core in range(NCORES):
        in_maps.append({
            "xt": _pack_xt_core(xt_all, core),
            "w": _pack_w_core(w, core),
            "bias": _pack_bias_core(bias, core),
        })

    res = run_bass_kernel_spmd(nc, in_maps, core_ids=list(range(NCORES)))
    _CACHE["last_results"] = res

    o_all = np.concatenate(
        [_unpack_core(res.results[core]["out"]) for core in range(NCORES)],
        axis=0)
    return _assemble(o_all, perm)


# revision 11
# speedup vs baseline: 1.0791x; 1.0791x over previous
"""Trainium2 Bass kernel for nn_ChannelWisePatchLevelObfuscator.

Math: split each (512,512) image into 32x32 patches of 16x16; per (channel,
group) apply a dense 256->256 obfuscation matmul over patch pixels (group =
(row+col) % 32), add bias, tanh, then permute channels.

Sharding: model-parallel over the 96 (channel, group) pairs -- 12 pairs per
core, each pair covering the FULL batch (T = B*NH = 2048 matmul rows). Unlike
batch-parallel sharding (which replicates all 12 MiB of fp16 weights on every
core), this loads each weight exactly once chip-wide: per-core traffic drops
from 36 MiB (12 x + 12 w + 12 out) to 25.5 MiB (12 x + 1.5 w + 12 out).

Layout strategy: the host packs x into a pair-major, contraction-major
("pixel on partition") slab layout and pre-permutes W/bias to match, so every
device DMA is a fully-contiguous [128 x 8KiB-per-partition] transfer at peak
HBM bandwidth. The channel permutation is applied for free during the host
unpack scatter.

Device loop per core: all weights (12 x [128,512] fp16 = 1.5 MiB) preloaded
into SBUF once. Per pair: one 1 MiB x DMA; per output half oc, 8 matmuls
(stationary W[kc,oc] streamed over 4 t-tiles of 512, K accumulated over 2
chunks) fill a 4-bank PSUM tile [128,2048]; ONE ScalarE activation then does
bias + tanh + PSUM->SBUF fp16 for the whole 2048-wide tile (4x fewer ACT
instructions than per-bank activations -- ScalarE was the covert near-
bottleneck of the batch-parallel kernel at 97/112 us busy).

Precision: matmul inputs and the tanh output are fp16 (fp32 PSUM accumulate);
rel err vs fp32 reference ~3.6e-4.
"""
import sys
import numpy as np

sys.path.insert(0, "/opt/trn_rl_repo")

import concourse.bacc as bacc  # noqa: E402
import concourse.mybir as mybir  # noqa: E402
import concourse.tile as tile  # noqa: E402
from concourse.bass_utils import run_bass_kernel_spmd  # noqa: E402

IMG, C, PS, G, B = 512, 3, 16, 32, 64
NH = NW = IMG // PS          # 32 patches per side
P2 = PS * PS                 # 256 pixels per patch
NCORES = 8
NPAIR = C * G                # 96 (channel, group) pairs
PPC = NPAIR // NCORES        # 12 pairs per core
TF = B * NH                  # 2048 matmul rows per pair (full batch)
NT = TF // 512               # 4 moving tiles of 512 per (kc, oc)

F32 = mybir.dt.float32
MM_DT = mybir.dt.float16     # matmul input dtype
OUT_DT = mybir.dt.float16    # device store dtype; host upcasts to fp32
NP_MM = np.float16

_g = np.arange(G)[:, None]
_r = np.arange(NH)[None, :]
COLS = (_g - _r) % NW        # (g, r) -> patch column belonging to group g

_CACHE = {}


def _build_nc():
    nc = bacc.Bacc("TRN2", target_bir_lowering=False, debug=False,
                   num_devices=NCORES)
    # slab layouts: per-pair x/out tiles are [128, 8 KiB-per-partition]
    # fully-contiguous transfers; w/bias load once up front.
    xt = nc.dram_tensor("xt", [PPC, 2, 128, TF], MM_DT, kind="ExternalInput")
    w = nc.dram_tensor("w", [PPC, 128, 512], MM_DT, kind="ExternalInput")
    bias = nc.dram_tensor("bias", [128, PPC * 2], F32, kind="ExternalInput")
    out = nc.dram_tensor("out", [PPC, 128, 2 * TF], OUT_DT,
                         kind="ExternalOutput")

    with tile.TileContext(nc) as tc:
        with tc.tile_pool(name="biasp", bufs=1) as bias_pool, \
             tc.tile_pool(name="wp", bufs=PPC) as w_pool, \
             tc.tile_pool(name="xtp", bufs=2 * PPC) as xt_pool, \
             tc.tile_pool(name="outp", bufs=8) as out_pool, \
             tc.tile_pool(name="psp", bufs=2, space="PSUM") as ps_pool:
            # Tile dependencies gate at whole-tile granularity, so weights
            # are PER-PAIR tiles and x is per-(pair, kc-half) tiles. ALL x
            # tiles stay resident (24 x 4 KiB/partition) so the load stream
            # is never slot-gated: it front-runs at full HBM rate while the
            # store ring is still empty, which keeps PE gap-free (HAM warm).
            # Everything inbound rides the SP ring in pair order (w[j] just
            # ahead of its x halves); the ACT ring carries only stores.
            bias_sb = bias_pool.tile([128, PPC * 2], F32)
            nc.sync.dma_start(bias_sb[:], bias[:, :])
            w_t = []
            xk = []
            for j in range(PPC):
                wt = w_pool.tile([128, 512], MM_DT)
                nc.sync.dma_start(wt[:], w[j])
                w_t.append(wt)
                for kc in range(2):
                    xt_t = xt_pool.tile([128, TF], MM_DT)
                    nc.sync.dma_start(xt_t[:], xt[j, kc])
                    xk.append(xt_t)
            for j in range(PPC):
                out_t = out_pool.tile([128, 2 * TF], OUT_DT)
                for oc in range(2):
                    ps = ps_pool.tile([128, TF], F32)  # 4 PSUM banks
                    for kc in range(2):
                        wof = kc * 256 + oc * 128
                        for tt in range(NT):
                            nc.tensor.matmul(
                                ps[:, tt * 512:(tt + 1) * 512],
                                w_t[j][:, wof:wof + 128],
                                xk[2 * j + kc][:, tt * 512:(tt + 1) * 512],
                                start=(kc == 0), stop=(kc == 1))
                    bidx = j * 2 + oc
                    nc.scalar.activation(
                        out_t[:, oc * TF:(oc + 1) * TF],
                        ps[:],
                        mybir.ActivationFunctionType.Tanh,
                        bias=bias_sb[:, bidx:bidx + 1],
                        scale=1.0)
                # store via SWDGE on the (idle) GpSimd engine: ScalarE stays
                # a pure ACTIVATE stream -- it is the serial mid-kernel pacer
                nc.gpsimd.dma_start(out[j], out_t[:])
    nc.compile()
    return nc


def _pack_xt_all(x):
    # (B, C, 512, 512) -> [96, 256, 2048] fp16: xt_all[pair, p, t] with
    # p = py*16+px on what becomes the partition axis, t = b*32 + r
    xp = x.reshape(B, C, NH, PS, NW, PS)               # b c r py cl px
    sel = xp[:, :, _r, :, COLS, :]                     # g r b c py px
    xt_all = sel.transpose(3, 0, 4, 5, 2, 1)           # c g py px b r
    return xt_all.reshape(NPAIR, P2, TF).astype(NP_MM)


def _pack_xt_core(xt_all, core):
    # [12, 256, 2048] -> [12, kc, 128, t] per-(pair, kc-half) slabs
    sl = xt_all[core * PPC:(core + 1) * PPC]
    return np.ascontiguousarray(sl.reshape(PPC, 2, 128, TF))


def _pack_w_core(w_full, core):
    # [c,g,p_in,p_out] -> [j, 128 k_lo, kc*256 + oc*128 + o_lo]
    sel = (w_full.reshape(NPAIR, P2, P2)[core * PPC:(core + 1) * PPC]
           .astype(NP_MM))
    sel = sel.reshape(PPC, 2, 128, 2, 128).transpose(0, 2, 1, 3, 4)
    return np.ascontiguousarray(sel.reshape(PPC, 128, 512))


def _pack_bias_core(b_full, core):
    # [c,g,o] -> [128 o_lo, j*2 + oc] fp32
    sel = b_full.reshape(NPAIR, P2)[core * PPC:(core + 1) * PPC]
    sel = sel.reshape(PPC, 2, 128).transpose(2, 0, 1)
    return np.ascontiguousarray(sel.reshape(128, PPC * 2))


def _unpack_core(out_dev):
    # [12, 128, oc*2048 + t] fp16 -> [12, 256, B, NH] f32 (o = oc*128+o_lo)
    od = out_dev.astype(np.float32).reshape(PPC, 128, 2, TF)
    return od.transpose(0, 2, 1, 3).reshape(PPC, P2, B, NH)


def _assemble(o_all, perm):
    # o_all [96, 256, B, NH] -> full (B, C, 512, 512) with channel perm
    src = (o_all.reshape(C, G, PS, PS, B, NH)
           .transpose(1, 5, 4, 0, 2, 3))               # g r b c py px
    tmp = np.empty((NH, NW, B, C, PS, PS), dtype=np.float32)
    tmp[_r, COLS] = src                                # tmp[r, (g-r)%32]
    img = tmp.transpose(2, 3, 0, 4, 1, 5).reshape(B, C, IMG, IMG)
    return np.ascontiguousarray(img[:, perm])


def kernel(x, obfuscation_weights, obfuscation_biases, channel_permutation):
    x = np.ascontiguousarray(x, dtype=np.float32)
    w = np.ascontiguousarray(obfuscation_weights, dtype=np.float32)
    bias = np.asarray(obfuscation_biases, dtype=np.float32)
    perm = np.asarray(channel_permutation, dtype=np.int64)

    if "nc" not in _CACHE:
        _CACHE["nc"] = _build_nc()
    nc = _CACHE["nc"]

    xt_all = _pack_xt_all(x)
    in_maps = []
    for core in range(NCORES):
        in_maps.append({
            "xt": _pack_xt_core(xt_all, core),
            "w": _pack_w_core(w, core),
            "bias": _pack_bias_core(bias, core),
        })

    res = run_bass_kernel_spmd(nc, in_maps, core_ids=list(range(NCORES)))
    _CACHE["last_results"] = res

    o_all = np.concatenate(
        [_unpack_core(res.results[core]["out"]) for core in range(NCORES)],
        axis=0)
    return _assemble(o_all, perm)
